# revision 45
# baseline (speedup 1.0000x reference)
"""Trainium2 Bass kernel for nn_AttentionBlock (B=16, C=512, H=W=32, 8 heads, d_k=64).

Sharding: data-parallel over batch; each of the 8 NeuronCores computes 2 batches.

All matmuls fp16 operands (1 cycle/row + fast weight load), fp32 PSUM accumulate.
Layout is fully transposed (channels on partitions) so no transposes are needed:
  qkT projection  : qpair[p] = [q_{2p}; q_{2p+1}]^T; kpad = k zero-padded to 128 rows
  v projection    : v_aug[:, t, h, 0:64] = v tokens, [..., 64:128] = 1.0
  attention       : scoresT = kpad.T @ qpair; expT = exp(scale*s - 4.5) (ACT -> fp16;
                    the constant shift keeps exp inside fp16 range, softmax-invariant)
                    res = [v|1..1].T @ expT: rows 0:64 = attn@v, rows 64:128 = sumexp
                    broadcast across 64 partitions -> recip + multiply directly from
                    PSUM (no DMA round trip, frees the PSUM bank ~3us earlier)
  out projection  : outT = W_out.T-chunks @ res_norm + (x + b_out + b_v@W_out)
                    (b_v never added to v: softmax rows sum to 1, so attn@(v+b_v)
                     = attn@v + b_v, and b_v@W_out is host-folded into xpb)

The attention phase has PE slack, so the next batch's projection matmuls and the
previous batch's output-projection matmuls are interleaved into the attention
emission order as filler work for the PE.

PSUM budget (8 banks): scores 3 x [128,512] + attnv res 4 x [128,512] +
filler accumulator 1 x [128,512].
"""
from collections import deque

import numpy as np

import concourse.bass as bass
from concourse import bacc
import concourse.mybir as mybir
import concourse.tile as tile
from concourse import bass_utils

F32 = mybir.dt.float32
F16 = mybir.dt.float16
AF = mybir.ActivationFunctionType
ALU = mybir.AluOpType

N_HEADS = 8
DK = 64
SCALE = DK ** -0.5
EXP_SHIFT = -4.5
C = 512
N = 1024            # tokens per batch (32*32)
NB = 2              # batches per core
NCORES = 8
NCH = C // 128      # 4 contraction chunks
NT = N // 128       # 8 token tiles
NPAIR = N_HEADS // 2


def build():
    nc = bacc.Bacc(None, target_bir_lowering=False, num_swdge_queues=4)
    x_d = nc.dram_tensor("x", (NB, C, N), F16, kind="ExternalInput")
    xpb_d = nc.dram_tensor("xpb", (NB, C, N), F16, kind="ExternalInput")
    wqk_d = nc.dram_tensor("w_qk", (C, 2, NPAIR, 128), F16, kind="ExternalInput")
    bqkt_d = nc.dram_tensor("b_qk_t", (128, 2, NPAIR), F32, kind="ExternalInput")
    wv_d = nc.dram_tensor("w_v", (C, C), F16, kind="ExternalInput")
    wout_d = nc.dram_tensor("w_out", (C, C), F16, kind="ExternalInput")
    zeros_d = nc.dram_tensor("zeros", (1, N), F16, kind="ExternalInput")
    bout_d = nc.dram_tensor("b_out_t", (128, NCH), F32, kind="ExternalInput")
    y_d = nc.dram_tensor("y", (NB, C, N), F16, kind="ExternalOutput")

    with tile.TileContext(nc) as tc:
        with (
            tc.tile_pool(name="const", bufs=1) as const,
            tc.tile_pool(name="persist", bufs=1) as persist,
            tc.tile_pool(name="sbwork", bufs=3) as sbwork,
            tc.tile_pool(name="sbexp", bufs=6) as sbexp,
            tc.tile_pool(name="ps_s", bufs=2, space="PSUM") as ps_s,
            tc.tile_pool(name="ps_res", bufs=3, space="PSUM") as ps_res,
            tc.tile_pool(name="ps_acc", bufs=1, space="PSUM") as ps_acc,
        ):
            # ---- weights / x staged in across 4 queues ----
            # kpad zero halves come from DRAM broadcast (cheap DMA) instead
            # of big DVE writes that would clog the early vector FIFO
            x_r = [persist.tile([128, N], F16, name=f"xr{ch}")
                   for ch in range(NCH)]
            qpair = [persist.tile([128, N], F16, name=f"qpair{p}")
                     for p in range(NPAIR)]
            kpad = [[persist.tile([128, N], F16, name=f"kpad{p}_{s}")
                     for s in range(2)] for p in range(NPAIR)]
            wqk = []
            for ch in range(NCH):
                w = const.tile([128, 2, NPAIR, 128], F16, name=f"wqk{ch}")
                [nc.sync, nc.scalar][ch % 2].dma_start(
                    w[:], wqk_d[ch * 128:(ch + 1) * 128])
                wqk.append(w)
            # pair-0 zero halves first: the very first score matmul reads them
            nc.sync.dma_start(kpad[0][0][64:128, :],
                              zeros_d[:].to_broadcast([64, N]))
            nc.scalar.dma_start(kpad[0][1][0:64, :],
                                zeros_d[:].to_broadcast([64, N]))
            # first-needed halves of x first (qkT nh=0, scores group 0),
            # then wv (v_unit(0) pops at step 0), then the nh=1 x halves
            for ch in range(NCH):
                nc.gpsimd.dma_start(
                    x_r[ch][:, 0:512], x_d[0, ch * 128:(ch + 1) * 128, 0:512])
            bqkt = const.tile([128, 2, NPAIR], F32)
            nc.sync.dma_start(bqkt[:], bqkt_d[:])
            bout_t = const.tile([128, NCH], F32)
            nc.scalar.dma_start(bout_t[:], bout_d[:])
            wv = []
            for ch in range(NCH):
                w2 = const.tile([128, C], F16, name=f"wv{ch}")
                nc.gpsimd.dma_start(w2[:], wv_d[ch * 128:(ch + 1) * 128, :])
                wv.append(w2)
            for ch in range(NCH):
                [nc.sync, nc.scalar, nc.sync, nc.scalar][ch].dma_start(
                    x_r[ch][:, 512:1024],
                    x_d[0, ch * 128:(ch + 1) * 128, 512:1024])
            for p in range(1, NPAIR):
                nc.sync.dma_start(kpad[p][0][64:128, :],
                                  zeros_d[:].to_broadcast([64, N]))
                nc.scalar.dma_start(kpad[p][1][0:64, :],
                                    zeros_d[:].to_broadcast([64, N]))

            # HAM warmup: dummy matmuls on memset data run during the initial
            # DMA wait so the real matmuls start at the full PE clock.
            warm = const.tile([128, 512], F16)
            nc.vector.memset(warm[:], 0.5)
            warm_ps = ps_acc.tile([128, 512], F32, tag="acc", name="warm_ps")
            for r in range(4):
                nc.tensor.matmul(warm_ps[:], warm[:, 0:128], warm[:],
                                 start=(r == 0), stop=(r == 3))
            # preload the exp table set during the DMA head (first real exp
            # would otherwise pay the ~2.7us ACT_TABLE_LOAD inside the stream)
            warm_exp = const.tile([128, 8], F16)
            nc.scalar.activation(out=warm_exp[:], in_=warm[:, 0:8],
                                 func=AF.Exp, scale=1.0)
            expbias = const.tile([128, 1], F32)
            nc.vector.memset(expbias[:], EXP_SHIFT)

            # ---- persistent per-batch buffers ----
            # v_aug cols 0:64 = 1.0 so attnv also emits sumexp broadcast
            # across partitions 0:64 of the PSUM result (partition-0-aligned
            # for the custom-DVE reciprocal); v lives in cols 64:128.
            # The 1MB of ones is written incrementally (t=0,1 here, the rest
            # inside batch-0 v_units) so it never clogs the early DVE FIFO.
            v_aug = persist.tile([128, NT, N_HEADS, 128], F16)
            nc.vector.memset(v_aug[:, 0:2, :, 0:DK], 1.0)
            res_all_db = [[persist.tile([128, N], F16, name=f"resall{bb}_{p}")
                           for p in range(NPAIR)] for bb in range(NB)]

            # ---- work units (closures) for PE-filler interleaving ----
            def xload_unit(b, ch):
                def f():
                    nc.gpsimd.dma_start(x_r[ch][:],
                                        x_d[b, ch * 128:(ch + 1) * 128, :])
                return f

            def qkT_unit(p, qk, nh):
                def f():
                    nsl = slice(nh * 512, nh * 512 + 512)
                    ps = ps_acc.tile([128, 512], F32, tag="acc", name="qk_ps")
                    for ch in range(NCH):
                        nc.tensor.matmul(
                            ps[:], wqk[ch][:, qk, p, :], x_r[ch][:, nsl],
                            start=(ch == 0), stop=(ch == NCH - 1))
                    if qk == 0:
                        nc.vector.tensor_scalar(
                            out=qpair[p][:, nsl], in0=ps[:],
                            scalar1=bqkt[:, 0, p:p + 1], scalar2=None,
                            op0=ALU.add)
                    else:
                        nc.vector.tensor_scalar(
                            out=kpad[p][0][0:64, nsl], in0=ps[0:64, :],
                            scalar1=bqkt[0:64, 1, p:p + 1], scalar2=None,
                            op0=ALU.add)
                        nc.vector.tensor_scalar(
                            out=kpad[p][1][64:128, nsl], in0=ps[64:128, :],
                            scalar1=bqkt[64:128, 1, p:p + 1], scalar2=None,
                            op0=ALU.add)
                return f

            def v_unit(t, first=False):
                def f():
                    if first and t >= 2:
                        nc.vector.memset(v_aug[:, t, :, 0:DK], 1.0)
                    ps = ps_acc.tile([128, 512], F32, tag="acc", name="v_ps")
                    for ch in range(NCH):
                        nc.tensor.matmul(
                            ps[:], x_r[ch][:, t * 128:(t + 1) * 128], wv[ch][:],
                            start=(ch == 0), stop=(ch == NCH - 1))
                    nc.vector.tensor_copy(
                        v_aug[:, t, :, DK:128],
                        ps[:].rearrange("p (h d) -> p h d", h=N_HEADS))
                return f

            def out_units(b, tail=False):
                xres = {}
                units = []

                def mk(ct, nh):
                    def f():
                        csl = slice(ct * 128, (ct + 1) * 128)
                        nsl = slice(nh * 512, nh * 512 + 512)
                        if b == 0 and ct not in xres:
                            xr = sbwork.tile([128, N], F16, tag="xres", bufs=4,
                                             name=f"x_res{b}_{ct}")
                            [nc.sync, nc.gpsimd][ct % 2].dma_start(
                                xr[:], xpb_d[0, csl, :])
                            xres[ct] = xr
                        if tail:
                            ps = ps_res.tile([128, 512], F32, tag="res",
                                             name="out_ps_t")
                        else:
                            ps = ps_acc.tile([128, 512], F32, tag="acc",
                                             name="out_ps")
                        for ch in range(NCH):
                            nc.tensor.matmul(
                                ps[:], wo[ch][:, csl],
                                res_all_db[b][ch][:, nsl],
                                start=(ch == 0), stop=(ch == NCH - 1))
                        out_sb = sbwork.tile([128, 512], F16, tag="out",
                                             name="out_sb")
                        if b == 0:
                            nc.vector.tensor_add(out_sb[:], ps[:],
                                                 xres[ct][:, nsl])
                        else:
                            # batch 1: x_r still holds its x; residual bias
                            # (b_out + b_v@W_out) is per-partition
                            nc.vector.tensor_scalar(
                                out=out_sb[:], in0=ps[:],
                                scalar1=bout_t[:, ct:ct + 1], scalar2=None,
                                op0=ALU.add)
                            nc.vector.tensor_add(out_sb[:], out_sb[:],
                                                 x_r[ct][:, nsl])
                        if tail:
                            # split the store across two hwdge queues: the
                            # last store is on the kernel's critical path
                            # (scalar is free, the exp stream has ended)
                            h0 = slice(nh * 512, nh * 512 + 256)
                            h1 = slice(nh * 512 + 256, nh * 512 + 512)
                            nc.sync.dma_start(y_d[b, csl, h0],
                                              out_sb[:, 0:256])
                            nc.scalar.dma_start(y_d[b, csl, h1],
                                                out_sb[:, 256:512])
                        else:
                            eng = [nc.sync, nc.gpsimd][ct % 2] if b == 0 \
                                else nc.sync
                            eng.dma_start(y_d[b, csl, nsl], out_sb[:])
                    return f

                for nh in range(2):
                    for ct in range(NCH):
                        units.append(mk(ct, nh))
                return units[:NCH], units[NCH:]

            filler = deque()

            def inject(k=1):
                for _ in range(min(k, len(filler))):
                    filler.popleft()()

            def norm_group(b, p, ic, res_ps):
                isl = slice(ic * 512, ic * 512 + 512)
                for s in range(2):
                    rcp = sbwork.tile([64, 512], F32, tag="rcp", bufs=2,
                                      name="rcp_sb")
                    nc.vector.reciprocal_approx_fast(
                        out=rcp[:], in_=res_ps[s][0:DK, :])
                    nc.vector.tensor_mul(
                        res_all_db[b][p][s * 64:(s + 1) * 64, isl],
                        res_ps[s][DK:128, :], rcp[:])

            # ---- emission schedule: one flat attention stream ----
            wo = []
            wo_units = []
            for ch in range(NCH):
                w = const.tile([128, C], F16, name=f"wout{ch}")
                wo.append(w)

                def mk_wo(ch=ch, w=w):
                    def f():
                        nc.sync.dma_start(w[:],
                                          wout_d[ch * 128:(ch + 1) * 128, :])
                    return f
                wo_units.append(mk_wo())

            def qkts(p):
                return [qkT_unit(p, qk, nh) for qk in range(2)
                        for nh in range(2)]

            with nc.named_scope("b0_proj"):
                # only the nh=0 halves up front; scores t=0..3 need just these.
                qkT_unit(0, 0, 0)()
                qkT_unit(0, 1, 0)()

            b0_nh0, b0_nh1 = out_units(0)
            b1_nh0, b1_nh1 = out_units(1, tail=True)
            b0_all = b0_nh0 + b0_nh1

            groups = [(b, p, ic) for b in range(NB) for p in range(NPAIR)
                      for ic in range(2)]
            group_fill = {
                0: [qkT_unit(0, 0, 1), qkT_unit(0, 1, 1)] + qkts(1),
                2: qkts(2), 3: qkts(3),
                4: [xload_unit(1, ch) for ch in range(NCH)],
                5: wo_units,
                6: qkts(0),                      # batch-1 weights from here
                8: qkts(1), 9: qkts(2), 10: qkts(3),
                11: b0_all[0:3], 12: b0_all[3:6], 13: b0_all[6:8],
                15: b1_nh0,
            }
            # v(b1, t) is legal only in the one step after attnv(b0,p3,ic1,t)
            # (WAR on v_aug) and before attnv(b1,p0,ic0,t) (RAW): pop 1:1
            # right after group 7's attnv emissions.
            group_post = {7: deque(v_unit(t) for t in range(NT))}
            pre = deque(v_unit(t, first=True)
                        for t in range(NT))   # batch-0 v, group 0

            nsteps = len(groups) * NT
            pending = deque()
            res_of = {}
            prev_exp = {}
            with nc.named_scope("attn_stream"):
                for k in range(nsteps + 2):
                    g, t = divmod(k, NT)
                    if k < nsteps:
                        b, p, ic = groups[g]
                        if t == 0:
                            if g in group_fill:
                                filler.extend(group_fill[g])
                            res_of[g] = [
                                ps_res.tile([128, 512], F32, tag="res",
                                            name=f"res{g}_{s}")
                                for s in range(2)]
                        isl = slice(ic * 512, ic * 512 + 512)
                        js = slice(t * 128, (t + 1) * 128)
                        s_ps = ps_s.tile([128, N], F32, tag="s", name="s_ps")
                        nc.tensor.matmul(s_ps[:, 0:512], kpad[p][0][:, js],
                                         qpair[p][:, isl],
                                         start=True, stop=True)
                        nc.tensor.matmul(s_ps[:, 512:1024], kpad[p][1][:, js],
                                         qpair[p][:, isl],
                                         start=True, stop=True)
                        exp_sb = sbexp.tile([128, 2, 512], F16, tag="exp",
                                            name="exp_sb")
                        nc.scalar.activation(out=exp_sb[:], in_=s_ps[:],
                                             func=AF.Exp, bias=expbias[:],
                                             scale=SCALE)
                        if g == 0 and pre:
                            pre.popleft()()
                        pending.append((g, t, exp_sb))
                    if len(pending) == 3 or (k >= nsteps and pending):
                        pg, pt, pexp = pending.popleft()
                        pb, pp, pic = groups[pg]

                        def attnv(s, spt, e, pg=pg, pp=pp):
                            nc.tensor.matmul(
                                res_of[pg][s][:],
                                v_aug[:, spt, 2 * pp + s, :], e[:, s, :],
                                start=(spt == 0), stop=(spt == NT - 1))

                        # s=1 lags one pop behind s=0 so the new group's
                        # second res-ring slot has a full extra step of WAR
                        # margin against the old group's norm reads
                        attnv(0, pt, pexp)
                        if pt >= 1:
                            attnv(1, pt - 1, prev_exp[pg])
                        prev_exp[pg] = pexp
                        if pt == NT - 1:
                            attnv(1, pt, pexp)
                            del prev_exp[pg]
                            norm_group(pb, pp, pic, res_of.pop(pg))
                        # group_post v_units overwrite v_aug[:, t]: pop with
                        # the same one-pop lag so the lagged s=1 attnv of the
                        # previous t has already been emitted
                        if pg in group_post and group_post[pg]:
                            if pt >= 1:
                                group_post[pg].popleft()()
                            if pt == NT - 1 and group_post[pg]:
                                group_post[pg].popleft()()
                        elif 2 <= t <= 6:
                            inject(1)

            with nc.named_scope("b1_out"):
                inject(len(filler))
                for u in b1_nh1:
                    u()

    nc.finalize()
    return nc


_NC = None


def _get_nc():
    global _NC
    if _NC is None:
        _NC = build()
    return _NC


def make_in_maps(x, W_qkv, b_qkv, W_out, b_out):
    x = np.ascontiguousarray(np.asarray(x, np.float32)).reshape(16, C, N)
    b_out = np.asarray(b_out, np.float32)
    w3 = np.asarray(W_qkv, np.float32).reshape(C, N_HEADS, 3, DK)
    w_qk = np.ascontiguousarray(
        np.stack([w3[:, :, 0], w3[:, :, 1]], axis=1).reshape(C, 2, NPAIR, 128))
    w_v = np.ascontiguousarray(w3[:, :, 2].reshape(C, C))
    b3 = np.asarray(b_qkv, np.float32).reshape(N_HEADS, 3, DK)
    b_qk_t = np.ascontiguousarray(
        np.stack([b3[:, 0], b3[:, 1]], axis=0)
        .reshape(2, NPAIR, 128).transpose(2, 0, 1))
    # b_v is never added to v on device: softmax rows sum to 1, so
    # attn@(v + b_v) = attn@v + b_v, and b_v@W_out folds into the residual.
    b_v = b3[:, 2].reshape(C)
    W_out = np.asarray(W_out, np.float32)
    bfull = b_out + b_v @ W_out
    b_out_t = np.ascontiguousarray(bfull.reshape(NCH, 128).T, np.float32)
    xpb = np.ascontiguousarray(x + bfull[None, :, None]).astype(np.float16)
    maps = []
    for core in range(NCORES):
        maps.append({
            "x": x[core * NB:(core + 1) * NB].astype(np.float16),
            "xpb": xpb[core * NB:(core + 1) * NB],
            "w_qk": w_qk.astype(np.float16),
            "b_qk_t": b_qk_t,
            "w_v": w_v.astype(np.float16),
            "w_out": W_out.astype(np.float16),
            "zeros": np.zeros((1, N), np.float16),
            "b_out_t": b_out_t,
        })
    return maps


def run_on_hw(in_maps, **kwargs):
    nc = _get_nc()
    return bass_utils.run_bass_kernel_spmd(
        nc, in_maps, core_ids=list(range(NCORES)), **kwargs)


def kernel(x, W_qkv, b_qkv, W_out, b_out):
    res = run_on_hw(make_in_maps(x, W_qkv, b_qkv, W_out, b_out))
    y = np.concatenate([r["y"] for r in res.results], axis=0)  # (16, C, N)
    return y.reshape(16, C, 32, 32).astype(np.float32)


# revision 46
# speedup vs baseline: 1.1946x; 1.1946x over previous
"""Trainium2 Bass kernel for nn_AttentionBlock (B=16, C=512, H=W=32, 8 heads, d_k=64).

Sharding: data-parallel over batch; each of the 8 NeuronCores computes 2 batches.

All matmuls fp16 operands (1 cycle/row + fast weight load), fp32 PSUM accumulate.
Layout is fully transposed (channels on partitions) so no transposes are needed:
  qkT projection  : qpair[p] = [q_{2p}; q_{2p+1}]^T; kpad = k zero-padded to 128 rows
  v projection    : v_aug[:, t, h, 0:64] = v tokens, [..., 64:128] = 1.0
  attention       : scoresT = kpad.T @ qpair; expT = exp(scale*s - 4.5) (ACT -> fp16;
                    the constant shift keeps exp inside fp16 range, softmax-invariant)
                    res = [v|1..1].T @ expT: rows 0:64 = attn@v, rows 64:128 = sumexp
                    broadcast across 64 partitions -> recip + multiply directly from
                    PSUM (no DMA round trip, frees the PSUM bank ~3us earlier)
  out projection  : outT = W_out.T-chunks @ res_norm + (x + b_out + b_v@W_out)
                    (b_v never added to v: softmax rows sum to 1, so attn@(v+b_v)
                     = attn@v + b_v, and b_v@W_out is host-folded into xpb)

The attention phase has PE slack, so the next batch's projection matmuls and the
previous batch's output-projection matmuls are interleaved into the attention
emission order as filler work for the PE.

PSUM budget (8 banks): scores 3 x [128,512] + attnv res 4 x [128,512] +
filler accumulator 1 x [128,512].
"""
from collections import deque

import numpy as np

import concourse.bass as bass
from concourse import bacc
import concourse.mybir as mybir
import concourse.tile as tile
from concourse import bass_utils

F32 = mybir.dt.float32
F16 = mybir.dt.float16
AF = mybir.ActivationFunctionType
ALU = mybir.AluOpType

N_HEADS = 8
DK = 64
SCALE = DK ** -0.5
EXP_SHIFT = -4.5
C = 512
N = 1024            # tokens per batch (32*32)
NB = 2              # batches per core
NCORES = 8
NCH = C // 128      # 4 contraction chunks
NT = N // 128       # 8 token tiles
NPAIR = N_HEADS // 2


def build():
    nc = bacc.Bacc(None, target_bir_lowering=False, num_swdge_queues=4)
    x_d = nc.dram_tensor("x", (NB, C, N), F16, kind="ExternalInput")
    xpb_d = nc.dram_tensor("xpb", (NB, C, N), F16, kind="ExternalInput")
    wqk_d = nc.dram_tensor("w_qk", (C, 2, NPAIR, 128), F16, kind="ExternalInput")
    bqkt_d = nc.dram_tensor("b_qk_t", (128, 2, NPAIR), F32, kind="ExternalInput")
    wv_d = nc.dram_tensor("w_v", (C, C), F16, kind="ExternalInput")
    wout_d = nc.dram_tensor("w_out", (C, C), F16, kind="ExternalInput")
    zeros_d = nc.dram_tensor("zeros", (1, N), F16, kind="ExternalInput")
    bout_d = nc.dram_tensor("b_out_t", (128, NCH), F32, kind="ExternalInput")
    y_d = nc.dram_tensor("y", (NB, C, N), F16, kind="ExternalOutput")

    with tile.TileContext(nc) as tc:
        with (
            tc.tile_pool(name="const", bufs=1) as const,
            tc.tile_pool(name="persist", bufs=1) as persist,
            tc.tile_pool(name="sbwork", bufs=3) as sbwork,
            tc.tile_pool(name="sbexp", bufs=6) as sbexp,
            tc.tile_pool(name="ps_s", bufs=2, space="PSUM") as ps_s,
            tc.tile_pool(name="ps_res", bufs=3, space="PSUM") as ps_res,
            tc.tile_pool(name="ps_acc", bufs=1, space="PSUM") as ps_acc,
        ):
            # ---- weights / x staged in across 4 queues ----
            # kpad zero halves come from DRAM broadcast (cheap DMA) instead
            # of big DVE writes that would clog the early vector FIFO
            x_r = [persist.tile([128, N], F16, name=f"xr{ch}")
                   for ch in range(NCH)]
            qpair = [persist.tile([128, N], F16, name=f"qpair{p}")
                     for p in range(NPAIR)]
            kpad = [[persist.tile([128, N], F16, name=f"kpad{p}_{s}")
                     for s in range(2)] for p in range(NPAIR)]
            wqk = []
            for ch in range(NCH):
                w = const.tile([128, 2, NPAIR, 128], F16, name=f"wqk{ch}")
                [nc.sync, nc.scalar][ch % 2].dma_start(
                    w[:], wqk_d[ch * 128:(ch + 1) * 128])
                wqk.append(w)
            # pair-0 zero halves first: the very first score matmul reads them
            nc.sync.dma_start(kpad[0][0][64:128, :],
                              zeros_d[:].to_broadcast([64, N]))
            nc.scalar.dma_start(kpad[0][1][0:64, :],
                                zeros_d[:].to_broadcast([64, N]))
            # first-needed halves of x first (qkT nh=0, scores group 0),
            # then wv (v_unit(0) pops at step 0), then the nh=1 x halves
            for ch in range(NCH):
                nc.gpsimd.dma_start(
                    x_r[ch][:, 0:512], x_d[0, ch * 128:(ch + 1) * 128, 0:512])
            bqkt = const.tile([128, 2, NPAIR], F32)
            nc.sync.dma_start(bqkt[:], bqkt_d[:])
            bout_t = const.tile([128, NCH], F32)
            nc.scalar.dma_start(bout_t[:], bout_d[:])
            wv = []
            for ch in range(NCH):
                w2 = const.tile([128, C], F16, name=f"wv{ch}")
                nc.gpsimd.dma_start(w2[:], wv_d[ch * 128:(ch + 1) * 128, :])
                wv.append(w2)
            for ch in range(NCH):
                [nc.sync, nc.scalar, nc.sync, nc.scalar][ch].dma_start(
                    x_r[ch][:, 512:1024],
                    x_d[0, ch * 128:(ch + 1) * 128, 512:1024])
            for p in range(1, NPAIR):
                nc.sync.dma_start(kpad[p][0][64:128, :],
                                  zeros_d[:].to_broadcast([64, N]))
                nc.scalar.dma_start(kpad[p][1][0:64, :],
                                    zeros_d[:].to_broadcast([64, N]))

            # HAM warmup: dummy matmuls on memset data run during the initial
            # DMA wait so the real matmuls start at the full PE clock.
            warm = const.tile([128, 512], F16)
            nc.vector.memset(warm[:], 0.5)
            warm_ps = ps_acc.tile([128, 512], F32, tag="acc", name="warm_ps")
            for r in range(4):
                nc.tensor.matmul(warm_ps[:], warm[:, 0:128], warm[:],
                                 start=(r == 0), stop=(r == 3))
            # preload the exp table set during the DMA head (first real exp
            # would otherwise pay the ~2.7us ACT_TABLE_LOAD inside the stream)
            warm_exp = const.tile([128, 8], F16)
            nc.scalar.activation(out=warm_exp[:], in_=warm[:, 0:8],
                                 func=AF.Exp, scale=1.0)
            expbias = const.tile([128, 1], F32)
            nc.vector.memset(expbias[:], EXP_SHIFT)

            # ---- persistent per-batch buffers ----
            # v_aug cols 0:64 = 1.0 so attnv also emits sumexp broadcast
            # across partitions 0:64 of the PSUM result (partition-0-aligned
            # for the custom-DVE reciprocal); v lives in cols 64:128.
            # The 1MB of ones is written incrementally (t=0,1 here, the rest
            # inside batch-0 v_units) so it never clogs the early DVE FIFO.
            v_aug = persist.tile([128, NT, N_HEADS, 128], F16)
            nc.vector.memset(v_aug[:, 0:2, :, 0:DK], 1.0)
            res_all_db = [[persist.tile([128, N], F16, name=f"resall{bb}_{p}")
                           for p in range(NPAIR)] for bb in range(NB)]

            # ---- work units (closures) for PE-filler interleaving ----
            def xload_unit(b, ch):
                def f():
                    nc.gpsimd.dma_start(x_r[ch][:],
                                        x_d[b, ch * 128:(ch + 1) * 128, :])
                return f

            def qkT_unit(p, qk, nh):
                def f():
                    nsl = slice(nh * 512, nh * 512 + 512)
                    ps = ps_acc.tile([128, 512], F32, tag="acc", name="qk_ps")
                    for ch in range(NCH):
                        nc.tensor.matmul(
                            ps[:], wqk[ch][:, qk, p, :], x_r[ch][:, nsl],
                            start=(ch == 0), stop=(ch == NCH - 1))
                    if qk == 0:
                        nc.vector.tensor_scalar(
                            out=qpair[p][:, nsl], in0=ps[:],
                            scalar1=bqkt[:, 0, p:p + 1], scalar2=None,
                            op0=ALU.add)
                    else:
                        nc.vector.tensor_scalar(
                            out=kpad[p][0][0:64, nsl], in0=ps[0:64, :],
                            scalar1=bqkt[0:64, 1, p:p + 1], scalar2=None,
                            op0=ALU.add)
                        nc.vector.tensor_scalar(
                            out=kpad[p][1][64:128, nsl], in0=ps[64:128, :],
                            scalar1=bqkt[64:128, 1, p:p + 1], scalar2=None,
                            op0=ALU.add)
                return f

            def v_unit(t, first=False):
                def f():
                    if first and t >= 2:
                        nc.vector.memset(v_aug[:, t, :, 0:DK], 1.0)
                    ps = ps_acc.tile([128, 512], F32, tag="acc", name="v_ps")
                    for ch in range(NCH):
                        nc.tensor.matmul(
                            ps[:], x_r[ch][:, t * 128:(t + 1) * 128], wv[ch][:],
                            start=(ch == 0), stop=(ch == NCH - 1))
                    nc.vector.tensor_copy(
                        v_aug[:, t, :, DK:128],
                        ps[:].rearrange("p (h d) -> p h d", h=N_HEADS))
                return f

            def out_units(b, tail=False):
                xres = {}
                units = []

                def mk(ct, nh):
                    def f():
                        csl = slice(ct * 128, (ct + 1) * 128)
                        nsl = slice(nh * 512, nh * 512 + 512)
                        if b == 0 and ct not in xres:
                            xr = sbwork.tile([128, N], F16, tag="xres", bufs=4,
                                             name=f"x_res{b}_{ct}")
                            [nc.sync, nc.gpsimd][ct % 2].dma_start(
                                xr[:], xpb_d[0, csl, :])
                            xres[ct] = xr
                        if tail:
                            ps = ps_res.tile([128, 512], F32, tag="res",
                                             name="out_ps_t")
                        else:
                            ps = ps_acc.tile([128, 512], F32, tag="acc",
                                             name="out_ps")
                        for ch in range(NCH):
                            nc.tensor.matmul(
                                ps[:], wo[ch][:, csl],
                                res_all_db[b][ch][:, nsl],
                                start=(ch == 0), stop=(ch == NCH - 1))
                        out_sb = sbwork.tile([128, 512], F16, tag="out",
                                             name="out_sb")
                        if b == 0:
                            nc.vector.tensor_add(out_sb[:], ps[:],
                                                 xres[ct][:, nsl])
                        else:
                            # batch 1: x_r still holds its x; residual bias
                            # (b_out + b_v@W_out) is per-partition
                            nc.vector.tensor_scalar(
                                out=out_sb[:], in0=ps[:],
                                scalar1=bout_t[:, ct:ct + 1], scalar2=None,
                                op0=ALU.add)
                            nc.vector.tensor_add(out_sb[:], out_sb[:],
                                                 x_r[ct][:, nsl])
                        if tail:
                            # split the store across two hwdge queues: the
                            # last store is on the kernel's critical path
                            # (scalar is free, the exp stream has ended)
                            h0 = slice(nh * 512, nh * 512 + 256)
                            h1 = slice(nh * 512 + 256, nh * 512 + 512)
                            nc.sync.dma_start(y_d[b, csl, h0],
                                              out_sb[:, 0:256])
                            nc.scalar.dma_start(y_d[b, csl, h1],
                                                out_sb[:, 256:512])
                        else:
                            eng = [nc.sync, nc.gpsimd][ct % 2] if b == 0 \
                                else nc.sync
                            eng.dma_start(y_d[b, csl, nsl], out_sb[:])
                    return f

                for nh in range(2):
                    for ct in range(NCH):
                        units.append(mk(ct, nh))
                return units[:NCH], units[NCH:]

            filler = deque()

            def inject(k=1):
                for _ in range(min(k, len(filler))):
                    filler.popleft()()

            def norm_group(b, p, ic, res_ps):
                isl = slice(ic * 512, ic * 512 + 512)
                for s in range(2):
                    rcp = sbwork.tile([64, 512], F32, tag="rcp", bufs=2,
                                      name="rcp_sb")
                    nc.vector.reciprocal_approx_fast(
                        out=rcp[:], in_=res_ps[s][0:DK, :])
                    nc.vector.tensor_mul(
                        res_all_db[b][p][s * 64:(s + 1) * 64, isl],
                        res_ps[s][DK:128, :], rcp[:])

            # ---- emission schedule: one flat attention stream ----
            wo = []
            wo_units = []
            for ch in range(NCH):
                w = const.tile([128, C], F16, name=f"wout{ch}")
                wo.append(w)

                def mk_wo(ch=ch, w=w):
                    def f():
                        nc.sync.dma_start(w[:],
                                          wout_d[ch * 128:(ch + 1) * 128, :])
                    return f
                wo_units.append(mk_wo())

            def qkts(p):
                return [qkT_unit(p, qk, nh) for qk in range(2)
                        for nh in range(2)]

            with nc.named_scope("b0_proj"):
                # only the nh=0 halves up front; scores t=0..3 need just these.
                qkT_unit(0, 0, 0)()
                qkT_unit(0, 1, 0)()

            b0_nh0, b0_nh1 = out_units(0)
            b1_nh0, b1_nh1 = out_units(1, tail=True)
            b0_all = b0_nh0 + b0_nh1

            groups = [(b, p, ic) for b in range(NB) for p in range(NPAIR)
                      for ic in range(2)]
            group_fill = {
                0: [qkT_unit(0, 0, 1), qkT_unit(0, 1, 1)] + qkts(1),
                2: qkts(2), 3: qkts(3),
                4: [xload_unit(1, ch) for ch in range(NCH)],
                5: wo_units,
                6: qkts(0),                      # batch-1 weights from here
                7: qkts(1), 9: qkts(2), 10: qkts(3),
                11: b0_all[0:3], 12: b0_all[3:6], 13: b0_all[6:8],
                15: b1_nh0,
            }
            # v(b1, t) is legal only in the one step after attnv(b0,p3,ic1,t)
            # (WAR on v_aug) and before attnv(b1,p0,ic0,t) (RAW): pop 1:1
            # right after group 7's attnv emissions.
            group_post = {7: deque(v_unit(t) for t in range(NT))}
            pre = deque(v_unit(t, first=True)
                        for t in range(NT))   # batch-0 v, group 0

            nsteps = len(groups) * NT
            pending = deque()
            res_of = {}
            prev_exp = {}
            with nc.named_scope("attn_stream"):
                for k in range(nsteps + 2):
                    g, t = divmod(k, NT)
                    if k < nsteps:
                        b, p, ic = groups[g]
                        if t == 0:
                            if g in group_fill:
                                filler.extend(group_fill[g])
                            res_of[g] = [
                                ps_res.tile([128, 512], F32, tag="res",
                                            name=f"res{g}_{s}")
                                for s in range(2)]
                        isl = slice(ic * 512, ic * 512 + 512)
                        js = slice(t * 128, (t + 1) * 128)
                        s_ps = ps_s.tile([128, N], F32, tag="s", name="s_ps")
                        nc.tensor.matmul(s_ps[:, 0:512], kpad[p][0][:, js],
                                         qpair[p][:, isl],
                                         start=True, stop=True)
                        nc.tensor.matmul(s_ps[:, 512:1024], kpad[p][1][:, js],
                                         qpair[p][:, isl],
                                         start=True, stop=True)
                        exp_sb = sbexp.tile([128, 2, 512], F16, tag="exp",
                                            name="exp_sb")
                        nc.scalar.activation(out=exp_sb[:], in_=s_ps[:],
                                             func=AF.Exp, bias=expbias[:],
                                             scale=SCALE)
                        if g == 0 and pre:
                            pre.popleft()()
                        pending.append((g, t, exp_sb))
                    if len(pending) == 3 or (k >= nsteps and pending):
                        pg, pt, pexp = pending.popleft()
                        pb, pp, pic = groups[pg]

                        def attnv(s, spt, e, pg=pg, pp=pp):
                            nc.tensor.matmul(
                                res_of[pg][s][:],
                                v_aug[:, spt, 2 * pp + s, :], e[:, s, :],
                                start=(spt == 0), stop=(spt == NT - 1))

                        # s=1 lags one pop behind s=0 so the new group's
                        # second res-ring slot has a full extra step of WAR
                        # margin against the old group's norm reads
                        attnv(0, pt, pexp)
                        if pt >= 1:
                            attnv(1, pt - 1, prev_exp[pg])
                        prev_exp[pg] = pexp
                        if pt == NT - 1:
                            attnv(1, pt, pexp)
                            del prev_exp[pg]
                            norm_group(pb, pp, pic, res_of.pop(pg))
                        # group_post v_units overwrite v_aug[:, t]: pop with
                        # the same one-pop lag so the lagged s=1 attnv of the
                        # previous t has already been emitted
                        if pg in group_post and group_post[pg]:
                            if pt >= 1:
                                group_post[pg].popleft()()
                            if pt == NT - 1 and group_post[pg]:
                                group_post[pg].popleft()()
                        elif 2 <= t <= 6:
                            inject(1)

            with nc.named_scope("b1_out"):
                inject(len(filler))
                for u in b1_nh1:
                    u()

    nc.finalize()
    return nc


_NC = None


def _get_nc():
    global _NC
    if _NC is None:
        _NC = build()
    return _NC


def make_in_maps(x, W_qkv, b_qkv, W_out, b_out):
    x = np.ascontiguousarray(np.asarray(x, np.float32)).reshape(16, C, N)
    b_out = np.asarray(b_out, np.float32)
    w3 = np.asarray(W_qkv, np.float32).reshape(C, N_HEADS, 3, DK)
    w_qk = np.ascontiguousarray(
        np.stack([w3[:, :, 0], w3[:, :, 1]], axis=1).reshape(C, 2, NPAIR, 128))
    w_v = np.ascontiguousarray(w3[:, :, 2].reshape(C, C))
    b3 = np.asarray(b_qkv, np.float32).reshape(N_HEADS, 3, DK)
    b_qk_t = np.ascontiguousarray(
        np.stack([b3[:, 0], b3[:, 1]], axis=0)
        .reshape(2, NPAIR, 128).transpose(2, 0, 1))
    # b_v is never added to v on device: softmax rows sum to 1, so
    # attn@(v + b_v) = attn@v + b_v, and b_v@W_out folds into the residual.
    b_v = b3[:, 2].reshape(C)
    W_out = np.asarray(W_out, np.float32)
    bfull = b_out + b_v @ W_out
    b_out_t = np.ascontiguousarray(bfull.reshape(NCH, 128).T, np.float32)
    xpb = np.ascontiguousarray(x + bfull[None, :, None]).astype(np.float16)
    maps = []
    for core in range(NCORES):
        maps.append({
            "x": x[core * NB:(core + 1) * NB].astype(np.float16),
            "xpb": xpb[core * NB:(core + 1) * NB],
            "w_qk": w_qk.astype(np.float16),
            "b_qk_t": b_qk_t,
            "w_v": w_v.astype(np.float16),
            "w_out": W_out.astype(np.float16),
            "zeros": np.zeros((1, N), np.float16),
            "b_out_t": b_out_t,
        })
    return maps


def run_on_hw(in_maps, **kwargs):
    nc = _get_nc()
    return bass_utils.run_bass_kernel_spmd(
        nc, in_maps, core_ids=list(range(NCORES)), **kwargs)


def kernel(x, W_qkv, b_qkv, W_out, b_out):
    res = run_on_hw(make_in_maps(x, W_qkv, b_qkv, W_out, b_out))
    y = np.concatenate([r["y"] for r in res.results], axis=0)  # (16, C, N)
    return y.reshape(16, C, 32, 32).astype(np.float32)
